# revision 2
# baseline (speedup 1.0000x reference)
"""Trainium2 Bass kernel for a 12-layer dense MLP autoencoder (512->...->5->...->512).

Strategy:
  - Pure data parallel: batch 131072 split as 16384 rows per NeuronCore (8 cores).
  - Activations kept feature-major ("transposed", [feat, batch]) on chip so every
    layer is out^T = relu(W^T @ a^T + b) with the contraction dim on partitions.
    The input x is transposed host-side (layout prep), so no on-device transposes.
  - The last layer swaps matmul operand roles (stationary = activations,
    moving = weights) to produce the natural [batch, feat] layout directly.
  - Matmuls run as float32r (full-rate fp32 path on the PE at N=512).
  - Bias+ReLU epilogues are single fused ops alternating between the Scalar (ACT)
    and Vector (DVE) engines to balance load.
  - Batch tiles of 512 are emitted pairwise layer-interleaved so the PE can work
    on tile B while tile A waits on an epilogue.
"""

import os
import sys
from contextlib import ExitStack

sys.path.insert(0, "/opt/trn_rl_repo")

import numpy as np

import concourse.bass as bass
import concourse.mybir as mybir
import concourse.tile as tile
from concourse import bacc
from concourse.bass_utils import run_bass_kernel_spmd

F32 = mybir.dt.float32

DIMS = [512, 256, 128, 64, 32, 16, 5, 16, 32, 64, 128, 256, 512]
N_LAYERS = 12
B = 131072
N_CORES = 8
BC = B // N_CORES  # 16384 rows per core
NB = 512  # batch tile (matmul free dim)
NT = BC // NB  # 32 batch tiles per core

MM_DTYPE = {
    "f32r": mybir.dt.float32r,
    "f32": mybir.dt.float32,
}[os.environ.get("MM_DTYPE", "f32r")]


def _ceil_div(a, b):
    return (a + b - 1) // b


def _chunks(n):
    """Split a dim into partition chunks of <=128."""
    return [min(128, n - i * 128) for i in range(_ceil_div(n, 128))]


def _wblob_layout():
    """Column offsets of each (layer, kchunk) weight block inside the packed
    [128, WCOLS] weight blob."""
    offs = {}
    col = 0
    for l in range(N_LAYERS):
        out_f = DIMS[l + 1]
        for ki, kc in enumerate(_chunks(DIMS[l])):
            offs[(l, ki)] = col
            col += out_f
    return offs, col


def _bblob_layout():
    """Column index of each (layer, mchunk) bias vector inside the packed
    [128, BCOLS] bias blob (layers 0..10 only; layer 11 handled separately)."""
    offs = {}
    col = 0
    for l in range(N_LAYERS - 1):
        for mo in range(len(_chunks(DIMS[l + 1]))):
            offs[(l, mo)] = col
            col += 1
    return offs, col


W_OFFS, WCOLS = _wblob_layout()
B_OFFS, BCOLS = _bblob_layout()


def build_bass():
    nc = bacc.Bacc("TRN2", target_bir_lowering=False, debug=False)

    MMD = MM_DTYPE
    xt_d = nc.dram_tensor("xt", [512, BC], MMD, kind="ExternalInput").ap()
    out_d = nc.dram_tensor("out", [BC, 512], F32, kind="ExternalOutput").ap()
    w_d = nc.dram_tensor("wblob", [128, WCOLS], MMD, kind="ExternalInput").ap()
    b_d = nc.dram_tensor("bblob", [128, BCOLS], F32, kind="ExternalInput").ap()
    b11_d = nc.dram_tensor("b11f", [128, 512], F32, kind="ExternalInput").ap()

    with ExitStack() as ctx:
        tc = ctx.enter_context(tile.TileContext(nc))
        wpool = ctx.enter_context(tc.tile_pool(name="weights", bufs=1))
        xpool = ctx.enter_context(tc.tile_pool(name="xin", bufs=3))
        hpool = ctx.enter_context(tc.tile_pool(name="acts", bufs=2))
        opool = ctx.enter_context(tc.tile_pool(name="outs", bufs=3))
        pspool = ctx.enter_context(tc.tile_pool(name="psum", bufs=2, space="PSUM"))

        w_sb = wpool.tile([128, WCOLS], MMD, tag="w", name="w_sb")
        nc.sync.dma_start(w_sb[:], w_d[:])
        b_sb = wpool.tile([128, BCOLS], F32, tag="b", name="b_sb")
        nc.sync.dma_start(b_sb[:], b_d[:])
        b11_sb = wpool.tile([128, 512], F32, tag="b11", name="b11_sb")
        nc.sync.dma_start(b11_sb[:], b11_d[:])

        def w_ap(l, ki, mo_off=0, mo_size=None):
            kc = _chunks(DIMS[l])[ki]
            out_f = DIMS[l + 1]
            if mo_size is None:
                mo_size = out_f
            col = W_OFFS[(l, ki)] + mo_off
            return w_sb[0:kc, col : col + mo_size]

        epi_ctr = [0]

        def epilogue(h_ap, ps_ap, bias_ap):
            """h = relu(ps + bias), alternating ACT / DVE."""
            if epi_ctr[0] % 2 == 0:
                nc.scalar.activation(
                    h_ap, ps_ap, mybir.ActivationFunctionType.Relu, bias=bias_ap
                )
            else:
                nc.vector.tensor_scalar(
                    h_ap,
                    ps_ap,
                    bias_ap,
                    0.0,
                    op0=mybir.AluOpType.add,
                    op1=mybir.AluOpType.max,
                )
            epi_ctr[0] += 1

        def emit_loads(n, st):
            acts = []
            for ki in range(4):
                t = xpool.tile([128, NB], MMD, tag=f"x{ki}", name=f"x_{n}_{ki}")
                nc.sync.dma_start(
                    t[:], xt_d[ki * 128 : (ki + 1) * 128, n * NB : (n + 1) * NB]
                )
                acts.append(t)
            st["acts"] = acts

        def emit_layer(n, l, st):
            in_f, out_f = DIMS[l], DIMS[l + 1]
            kcs = _chunks(in_f)
            par = n % 2
            if l < N_LAYERS - 1:
                mcs = _chunks(out_f)
                outs = []
                for mo, msize in enumerate(mcs):
                    ps = pspool.tile(
                        [msize, NB], F32, tag=f"ps{par}", name=f"ps_{n}_{l}_{mo}"
                    )
                    for ki, kc in enumerate(kcs):
                        lhsT = w_ap(l, ki, mo * 128, msize)
                        rhs = st["acts"][ki][:]
                        nc.tensor.matmul(
                            ps[:],
                            lhsT,
                            rhs,
                            start=(ki == 0),
                            stop=(ki == len(kcs) - 1),
                        )
                    h = hpool.tile([msize, NB], MMD, tag=f"h{l}_{mo}", name=f"h_{n}_{l}_{mo}")
                    bcol = B_OFFS[(l, mo)]
                    epilogue(h[:], ps[:], b_sb[0:msize, bcol : bcol + 1])
                    outs.append(h)
                st["acts"] = outs
            else:
                # Last layer: natural-layout output. stationary = h11^T slice
                # [128 feat, 128 batch], moving = w11 chunk [128 feat, 512 out].
                for bi in range(4):
                    ps = pspool.tile([128, NB], F32, tag=f"po{par}", name=f"po_{n}_{bi}")
                    for ki, kc in enumerate(kcs):
                        lhsT = st["acts"][ki][:, bi * 128 : (bi + 1) * 128]
                        rhs = w_ap(l, ki)
                        nc.tensor.matmul(
                            ps[:], lhsT, rhs, start=(ki == 0), stop=(ki == len(kcs) - 1)
                        )
                    tmp = opool.tile([128, NB], F32, tag=f"tmp{par}", name=f"tmp_{n}_{bi}")
                    nc.vector.tensor_add(tmp[:], ps[:], b11_sb[:])
                    ot = opool.tile([128, NB], F32, tag=f"ot{par}", name=f"ot_{n}_{bi}")
                    nc.scalar.activation(ot[:], tmp[:], mybir.ActivationFunctionType.Relu)
                    row = n * NB + bi * 128
                    nc.sync.dma_start(out_d[row : row + 128, :], ot[:])

        # Pairwise layer-interleaved emission over the 32 batch tiles.
        for p in range(NT // 2):
            pair = (2 * p, 2 * p + 1)
            states = {}
            for n in pair:
                states[n] = {}
                emit_loads(n, states[n])
            for l in range(N_LAYERS):
                for n in pair:
                    emit_layer(n, l, states[n])

    nc.compile()
    return nc


_NC_CACHE = None


def _get_nc():
    global _NC_CACHE
    if _NC_CACHE is None:
        _NC_CACHE = build_bass()
    return _NC_CACHE


def prep_in_maps(x, ws, bs):
    """Build per-core input maps from full inputs (host-side layout prep)."""
    wblob = np.zeros((128, WCOLS), np.float32)
    for l in range(N_LAYERS):
        out_f = DIMS[l + 1]
        for ki, kc in enumerate(_chunks(DIMS[l])):
            col = W_OFFS[(l, ki)]
            wblob[:kc, col : col + out_f] = ws[l][ki * 128 : ki * 128 + kc, :]
    bblob = np.zeros((128, BCOLS), np.float32)
    for l in range(N_LAYERS - 1):
        for mo, msize in enumerate(_chunks(DIMS[l + 1])):
            bblob[:msize, B_OFFS[(l, mo)]] = bs[l][mo * 128 : mo * 128 + msize]
    b11f = np.ascontiguousarray(
        np.broadcast_to(bs[11].astype(np.float32), (128, 512))
    )

    in_maps = []
    for c in range(N_CORES):
        xt_c = np.ascontiguousarray(x[c * BC : (c + 1) * BC, :].T)
        in_maps.append({"xt": xt_c, "wblob": wblob, "bblob": bblob, "b11f": b11f})
    return in_maps


def kernel(**inputs):
    x = np.asarray(inputs["x"], dtype=np.float32)
    ws = [np.asarray(inputs[f"w{i}"], dtype=np.float32) for i in range(N_LAYERS)]
    bs = [np.asarray(inputs[f"b{i}"], dtype=np.float32) for i in range(N_LAYERS)]

    nc = _get_nc()
    in_maps = prep_in_maps(x, ws, bs)
    res = run_bass_kernel_spmd(nc, in_maps, core_ids=list(range(N_CORES)))
    return np.concatenate([res.results[c]["out"] for c in range(N_CORES)], axis=0)


# revision 8
# speedup vs baseline: 22.4004x; 22.4004x over previous
"""Trainium2 Bass kernel for a 12-layer dense MLP autoencoder (512->...->5->...->512).

Strategy:
  - Pure data parallel: batch 131072 split as 16384 rows per NeuronCore (8 cores).
  - Activations kept feature-major ("transposed", [feat, batch]) on chip so every
    layer is out^T = relu(W^T @ a^T + b) with the contraction dim on partitions.
    The input x is transposed + tile-packed host-side, so no on-device transposes
    and one 1MB load DMA per 512-row batch tile.
  - The last layer swaps matmul operand roles (stationary = activations,
    moving = weights) to produce the natural [batch, feat] layout directly;
    its bias is folded into the matmul as a K=1 rank-1 update (ones x b11),
    so the epilogue is a single relu.
  - Matmuls run as float32r (full-rate fp32 path on the PE at N=512).
  - Middle layers (64/32/16/5-wide) are packed 4 batch-tiles deep into the
    128x128 PE array via tile_position (row/col array tiling), with weights
    host-stacked at the matching partition offsets. One fused bias+relu
    epilogue covers all 4 batch tiles.
  - Epilogues alternate between the Scalar (ACT) and Vector (DVE) engines.
  - Two groups of 4 batch tiles are emitted step-interleaved so the PE can
    work on group B while group A waits on an epilogue.
"""

import os
import sys
from contextlib import ExitStack

sys.path.insert(0, "/opt/trn_rl_repo")

import numpy as np

import concourse.bass as bass
import concourse.mybir as mybir
import concourse.tile as tile
from concourse import bacc
from concourse.bass_utils import run_bass_kernel_spmd

F32 = mybir.dt.float32
BF16 = mybir.dt.bfloat16

DIMS = [512, 256, 128, 64, 32, 16, 5, 16, 32, 64, 128, 256, 512]
N_LAYERS = 12
B = 131072
N_CORES = 8
BC = B // N_CORES  # 16384 rows per core
NB = 512  # batch tile (matmul free dim)
NT = BC // NB  # 32 batch tiles per core
GS = 4  # batch tiles per pack-group

MM_DTYPE = {"f32r": mybir.dt.float32r, "f32": F32}[os.environ.get("MM_DTYPE", "f32r")]
BF16_IN = os.environ.get("BF16_IN", "1") == "1"
BF16_OUT = os.environ.get("BF16_OUT", "0") == "1"
PACK_MID = os.environ.get("PACK_MID", "1") == "1"

# stacking factor for packed middle-layer weights (copies along partitions)
STACKS = {3: 2, 4: 4, 5: 4, 6: 4, 7: 4, 8: 4, 9: 2}


def _ceil_div(a, b):
    return (a + b - 1) // b


def _chunks(n):
    return [min(128, n - i * 128) for i in range(_ceil_div(n, 128))]


def _wblob_layout():
    """Columns in the packed [128, WCOLS] weight blob: plain per-(l,ki) chunks,
    partition-stacked copies for packed middle layers, the ones row and the
    b11 row for the last layer's fused bias."""
    offs = {}
    col = 0
    for l in range(N_LAYERS):
        out_f = DIMS[l + 1]
        for ki, _ in enumerate(_chunks(DIMS[l])):
            offs[("w", l, ki)] = col
            col += out_f
    for l in STACKS:
        offs[("ws", l)] = col
        col += DIMS[l + 1]
    offs[("ones",)] = col
    col += 128
    offs[("b11row",)] = col
    col += 512
    return offs, col


def _bblob_layout():
    offs = {}
    col = 0
    for l in range(N_LAYERS - 1):
        for mo in range(len(_chunks(DIMS[l + 1]))):
            offs[("b", l, mo)] = col
            col += 1
    for l in (2, 3, 4, 5, 6, 7, 8):
        offs[("bs", l)] = col
        col += 1
    return offs, col


def _w16blob_layout():
    """bf16 weight blob: layer-0 chunks plus the col-tiled middle layers."""
    offs = {}
    col = 0
    for ki in range(4):
        offs[("w0", ki)] = col
        col += 256
    offs[("w2",)] = col
    col += 64
    for l in (3, 4, 5, 6, 7, 8):
        offs[("ws", l)] = col
        col += DIMS[l + 1]
    return offs, col


W_OFFS, WCOLS = _wblob_layout()
B_OFFS, BCOLS = _bblob_layout()
W16_OFFS, W16COLS = _w16blob_layout()


def build_bass():
    nc = bacc.Bacc("TRN2", target_bir_lowering=False, debug=False)

    MMD = MM_DTYPE
    IN_D = BF16 if BF16_IN else MMD
    OUT_D = BF16 if BF16_OUT else F32

    xt_d = nc.dram_tensor("xt", [128, NT * GS * NB], IN_D, kind="ExternalInput").ap()
    out_d = nc.dram_tensor("out", [BC, 512], OUT_D, kind="ExternalOutput").ap()
    w_d = nc.dram_tensor("wblob", [128, WCOLS], MMD, kind="ExternalInput").ap()
    b_d = nc.dram_tensor("bblob", [128, BCOLS], F32, kind="ExternalInput").ap()
    w16_d = nc.dram_tensor("w16", [128, W16COLS], BF16, kind="ExternalInput").ap()

    # view of out as [tile n][sbuf partition p][bi chunk][col]
    out_v = out_d.rearrange("(n bi p) c -> n p bi c", bi=GS, p=128)

    with ExitStack() as ctx:
        tc = ctx.enter_context(tile.TileContext(nc))
        wpool = ctx.enter_context(tc.tile_pool(name="weights", bufs=1))
        xpool = ctx.enter_context(tc.tile_pool(name="xin", bufs=1))
        hpool = ctx.enter_context(tc.tile_pool(name="acts", bufs=1))
        opool = ctx.enter_context(tc.tile_pool(name="outs", bufs=1))
        pspool = ctx.enter_context(tc.tile_pool(name="psum", bufs=1, space="PSUM"))

        w_sb = wpool.tile([128, WCOLS], MMD, tag="w", name="w_sb")
        nc.sync.dma_start(w_sb[:], w_d[:])
        b_sb = wpool.tile([128, BCOLS], F32, tag="b", name="b_sb")
        nc.sync.dma_start(b_sb[:], b_d[:])
        w16_sb = wpool.tile([128, W16COLS], BF16, tag="w16", name="w16_sb")
        nc.sync.dma_start(w16_sb[:], w16_d[:])

        def w_ap(l, ki, mo_off=0, mo_size=None):
            kc = _chunks(DIMS[l])[ki]
            out_f = DIMS[l + 1]
            if mo_size is None:
                mo_size = out_f
            if l == 0 and BF16_IN:
                col = W16_OFFS[("w0", ki)] + mo_off
                return w16_sb[0:kc, col : col + mo_size]
            if l == 2 and PACK_MID:
                col = W16_OFFS[("w2",)] + mo_off
                return w16_sb[0:kc, col : col + mo_size]
            col = W_OFFS[("w", l, ki)] + mo_off
            return w_sb[0:kc, col : col + mo_size]

        def ws_ap(l, p0, psz):
            # col-tiled middle layers (3..8) need bf16 weights; L9 (row tiling
            # only) stays in the f32r blob.
            if l == 9:
                col = W_OFFS[("ws", 9)]
                return w_sb[p0 : p0 + psz, col : col + DIMS[10]]
            col = W16_OFFS[("ws", l)]
            return w16_sb[p0 : p0 + psz, col : col + DIMS[l + 1]]

        def bias_ap(key, p0=0, psz=128):
            col = B_OFFS[key]
            return b_sb[p0 : p0 + psz, col : col + 1]

        XB_BUFS = 8 if BF16_IN else 6
        OB_BUFS = 3

        epi_ctr = [0]

        def epilogue(h_ap, ps_ap, b_ap):
            """h = relu(ps + bias) (bias optional), alternating ACT / DVE."""
            if epi_ctr[0] % 2 == 0:
                nc.scalar.activation(
                    h_ap,
                    ps_ap,
                    mybir.ActivationFunctionType.Relu,
                    bias=b_ap if b_ap is not None else 0.0,
                )
            else:
                if b_ap is not None:
                    nc.vector.tensor_scalar(
                        h_ap, ps_ap, b_ap, 0.0,
                        op0=mybir.AluOpType.add, op1=mybir.AluOpType.max,
                    )
                else:
                    nc.vector.tensor_scalar_max(h_ap, ps_ap, 0.0)
            epi_ctr[0] += 1

        def mm(ps_ap, lhsT, rhs, start, stop, tp=None):
            nc.tensor.matmul(ps_ap, lhsT, rhs, start=start, stop=stop, tile_position=tp)

        def step_loads(g, st):
            st["xb"] = []
            for j in range(GS):
                n = GS * g + j
                t = xpool.tile([128, GS * NB], IN_D, tag="xb", bufs=XB_BUFS, name=f"x_{n}")
                nc.sync.dma_start(t[:], xt_d[:, n * GS * NB : (n + 1) * GS * NB])
                st["xb"].append(t)

        def step_head2(g, jpair, st):
            """L0 + L1 for two batch tiles; accumulation chains from different
            PSUM banks are emitted round-robin so each chain's next matmul
            never waits on its own bank's drain."""
            units = []  # (psum, h1-list, n, mo)
            for j in jpair:
                n = GS * g + j
                st.setdefault("h1", {})[j] = []
                for mo in range(2):
                    ps = pspool.tile([128, NB], F32, tag="ps", bufs=4,
                                     name=f"ps0_{n}_{mo}")
                    units.append((ps, j, n, mo))
            for ki in range(4):
                for ps, j, n, mo in units:
                    mm(ps[:], w_ap(0, ki, mo * 128, 128),
                       st["xb"][j][:, ki * NB : (ki + 1) * NB], ki == 0, ki == 3)
            for ps, j, n, mo in units:
                h = hpool.tile([128, NB], MMD, tag=f"h1_{mo}", bufs=4,
                               name=f"h1_{n}_{mo}")
                epilogue(h[:], ps[:], bias_ap(("b", 0, mo)))
                st["h1"][j].append(h)
            l1units = []
            for j in jpair:
                n = GS * g + j
                ps = pspool.tile([128, NB], F32, tag="ps", bufs=4, name=f"ps1_{n}")
                l1units.append((ps, j))
            for ki in range(2):
                for ps, j in l1units:
                    mm(ps[:], w_ap(1, ki), st["h1"][j][ki][:], ki == 0, ki == 1)
            h2d = BF16 if PACK_MID else MMD
            for ps, j in l1units:
                n = GS * g + j
                h2 = hpool.tile([128, NB], h2d, tag="h2", bufs=5, name=f"h2_{n}")
                epilogue(h2[:], ps[:], bias_ap(("b", 1, 0)))
                st.setdefault("h2", {})[j] = h2

        def step_l2(g, p, st):
            """L2 (128->64): pack 2 batch tiles into the two column halves."""
            ja, jb = 2 * p, 2 * p + 1
            ps = pspool.tile([128, NB], F32, tag="ps", bufs=4, name=f"ps2_{g}_{p}")
            mm(ps[0:64, :], w_ap(2, 0), st["h2"][ja][:], True, True, tp=(0, 0))
            mm(ps[64:128, :], w_ap(2, 0), st["h2"][jb][:], True, True, tp=(0, 64))
            h3 = hpool.tile([128, NB], BF16, tag="h3p", bufs=4, name=f"h3p_{g}_{p}")
            epilogue(h3[:], ps[:], bias_ap(("bs", 2)))
            st.setdefault("h3p", {})[p] = h3

        def step_mid(g, l, st):
            """L3..L7 packed: 4 batch tiles at 32-partition offsets."""
            kin, kout = DIMS[l], DIMS[l + 1]
            src_key = {3: "h3p", 4: "h4", 5: "h5", 6: "h6", 7: "h7"}[l]
            dst_key = {3: "h4", 4: "h5", 5: "h6", 6: "h7", 7: "h8"}[l]
            ps = pspool.tile([128, NB], F32, tag="ps", bufs=4, name=f"psm_{g}_{l}")
            for j in range(GS):
                if l == 3:
                    src = st["h3p"][j // 2]
                    rb = 64 * (j % 2)
                else:
                    src = st[src_key]
                    rb = 32 * j
                mm(ps[32 * j : 32 * j + kout, :], ws_ap(l, rb, kin),
                   src[rb : rb + kin, :], True, True, tp=(rb, 32 * j))
            h = hpool.tile([128, NB], BF16, tag=dst_key, bufs=2, name=f"{dst_key}_{g}")
            epilogue(h[:], ps[:], bias_ap(("bs", l)))
            st[dst_key] = h

        def step_l8(g, st):
            """L8 (32->64): two passes of 2 tiles into column halves."""
            st["h9p"] = {}
            for p in range(2):
                ja, jb = 2 * p, 2 * p + 1
                ps = pspool.tile([128, NB], F32, tag="ps", bufs=4, name=f"ps8_{g}_{p}")
                mm(ps[0:64, :], ws_ap(8, 32 * ja, 32), st["h8"][32 * ja : 32 * ja + 32, :],
                   True, True, tp=(32 * ja, 0))
                mm(ps[64:128, :], ws_ap(8, 32 * jb, 32), st["h8"][32 * jb : 32 * jb + 32, :],
                   True, True, tp=(32 * jb, 64))
                h9 = hpool.tile([128, NB], MMD, tag="h9p", bufs=4, name=f"h9p_{g}_{p}")
                epilogue(h9[:], ps[:], bias_ap(("bs", 8)))
                st["h9p"][p] = h9

        def step_l9(g, st):
            """L9 (64->128): one full-width matmul per batch tile."""
            st["h10"] = {}
            for j in range(GS):
                src = st["h9p"][j // 2]
                rb = 64 * (j % 2)
                ps = pspool.tile([128, NB], F32, tag="ps", bufs=4, name=f"ps9_{g}_{j}")
                mm(ps[:], ws_ap(9, rb, 64), src[rb : rb + 64, :], True, True, tp=(rb, 0))
                h10 = hpool.tile([128, NB], MMD, tag="h10", bufs=8, name=f"h10_{g}_{j}")
                epilogue(h10[:], ps[:], bias_ap(("b", 9, 0)))
                st["h10"][j] = h10

        # --- unpacked fallback for the middle layers -------------------------
        def step_mid_plain(g, l, st):
            src_key = {2: "h2", 3: "h3", 4: "h4", 5: "h5", 6: "h6", 7: "h7",
                       8: "h8", 9: "h9"}[l]
            dst_key = {2: "h3", 3: "h4", 4: "h5", 5: "h6", 6: "h7", 7: "h8",
                       8: "h9", 9: "h10"}[l]
            kin, kout = DIMS[l], DIMS[l + 1]
            st.setdefault(dst_key, {})
            for j in range(GS):
                src = st[src_key][j] if l > 2 else st["h2"][j]
                ps = pspool.tile([kout, NB], F32, tag="ps", bufs=4, name=f"psm_{g}_{l}_{j}")
                mm(ps[:], w_ap(l, 0), src[0:kin, :], True, True)
                h = hpool.tile([kout, NB], MMD, tag=f"{dst_key}_{j}", bufs=2,
                               name=f"{dst_key}_{g}_{j}")
                epilogue(h[:], ps[:], bias_ap(("b", l, 0), 0, kout))
                st[dst_key][j] = h

        def step_tail(g, j, st):
            """L10 + L11 + store for batch tile j of group g."""
            n = GS * g + j
            h10 = st["h10"][j]
            h11 = []
            for mo in range(2):
                ps = pspool.tile([128, NB], F32, tag="ps", bufs=4, name=f"ps10_{n}_{mo}")
                mm(ps[:], w_ap(10, 0, mo * 128, 128), h10[:], True, True)
                h = hpool.tile([128, NB], MMD, tag=f"h11_{mo}", bufs=4, name=f"h11_{n}_{mo}")
                epilogue(h[:], ps[:], bias_ap(("b", 10, mo)))
                h11.append(h)
            ones = w_sb[0:1, W_OFFS[("ones",)] : W_OFFS[("ones",)] + 128]
            b11r = w_sb[0:1, W_OFFS[("b11row",)] : W_OFFS[("b11row",)] + 512]
            ob = opool.tile([128, GS * NB], OUT_D, tag="ob", bufs=OB_BUFS, name=f"ob_{n}")
            pos = [
                pspool.tile([128, 2 * NB], F32, tag="po", bufs=2, name=f"po_{n}_{h}")
                for h in range(2)
            ]
            slices = [(pos[bi // 2], pos[bi // 2][:, (bi % 2) * NB : (bi % 2 + 1) * NB])
                      for bi in range(GS)]
            for ki in range(2):
                for bi, (_, sl) in enumerate(slices):
                    mm(sl, h11[ki][:, bi * 128 : (bi + 1) * 128], w_ap(11, ki),
                       ki == 0, False)
            for bi, (_, sl) in enumerate(slices):
                mm(sl, ones, b11r, False, True)
            for half in range(2):
                epilogue(ob[:, half * 2 * NB : (half + 1) * 2 * NB], pos[half][:], None)
            nc.sync.dma_start(
                out_v[n], ob[:].rearrange("p (bi c) -> p bi c", bi=GS)
            )

        def group_steps(g, st):
            yield lambda: step_loads(g, st)
            if PACK_MID:
                yield lambda: step_head2(g, (0, 1), st)
                yield lambda: step_l2(g, 0, st)
                yield lambda: step_head2(g, (2, 3), st)
                yield lambda: step_l2(g, 1, st)
                for l in (3, 4, 5, 6, 7):
                    yield lambda l=l: step_mid(g, l, st)
                yield lambda: step_l8(g, st)
                yield lambda: step_l9(g, st)
            else:
                yield lambda: step_head2(g, (0, 1), st)
                yield lambda: step_head2(g, (2, 3), st)
                for l in (2, 3, 4, 5, 6, 7, 8, 9):
                    yield lambda l=l: step_mid_plain(g, l, st)
            for j in range(GS):
                yield lambda j=j: step_tail(g, j, st)

        def emit_body():
            n_groups = NT // GS
            for gp in range(n_groups // 2):
                ga, gb = 2 * gp, 2 * gp + 1
                sta, stb = {}, {}
                for sa, sb in zip(group_steps(ga, sta), group_steps(gb, stb)):
                    sa()
                    sb()

        repeat = int(os.environ.get("REPEAT", "1"))
        if repeat > 1:
            with tc.For_i(0, repeat, 1):
                emit_body()
        else:
            emit_body()

    nc.compile()
    return nc


_NC_CACHE = None


def _get_nc():
    global _NC_CACHE
    if _NC_CACHE is None:
        _NC_CACHE = build_bass()
    return _NC_CACHE


def prep_in_maps(x, ws, bs):
    """Build per-core input maps from full inputs (host-side layout prep)."""
    import ml_dtypes

    np_in = ml_dtypes.bfloat16 if BF16_IN else np.float32

    wblob = np.zeros((128, WCOLS), np.float32)
    for l in range(N_LAYERS):
        out_f = DIMS[l + 1]
        for ki, kc in enumerate(_chunks(DIMS[l])):
            col = W_OFFS[("w", l, ki)]
            wblob[:kc, col : col + out_f] = ws[l][ki * 128 : ki * 128 + kc, :]
    for l, reps in STACKS.items():
        kin, kout = DIMS[l], DIMS[l + 1]
        col = W_OFFS[("ws", l)]
        for r in range(reps):
            p0 = r * (64 if reps == 2 else 32)
            wblob[p0 : p0 + kin, col : col + kout] = ws[l]
    wblob[0, W_OFFS[("ones",)] : W_OFFS[("ones",)] + 128] = 1.0
    wblob[0, W_OFFS[("b11row",)] : W_OFFS[("b11row",)] + 512] = bs[11]

    bblob = np.zeros((128, BCOLS), np.float32)
    for l in range(N_LAYERS - 1):
        for mo, msize in enumerate(_chunks(DIMS[l + 1])):
            bblob[:msize, B_OFFS[("b", l, mo)]] = bs[l][mo * 128 : mo * 128 + msize]
    for l in (2, 3, 4, 5, 6, 7, 8):
        kout = DIMS[l + 1]
        col = B_OFFS[("bs", l)]
        if kout == 64:  # 2-way stack at 0/64
            bblob[0:64, col] = bs[l]
            bblob[64:128, col] = bs[l]
        else:  # 4-way stack at 32-offsets
            for r in range(4):
                bblob[32 * r : 32 * r + kout, col] = bs[l]

    w16 = np.zeros((128, W16COLS), ml_dtypes.bfloat16)
    for ki in range(4):
        col = W16_OFFS[("w0", ki)]
        w16[:, col : col + 256] = ws[0][ki * 128 : (ki + 1) * 128, :].astype(
            ml_dtypes.bfloat16
        )
    col = W16_OFFS[("w2",)]
    w16[:, col : col + 64] = ws[2].astype(ml_dtypes.bfloat16)
    for l in (3, 4, 5, 6, 7, 8):
        kin, kout = DIMS[l], DIMS[l + 1]
        reps = STACKS[l]
        col = W16_OFFS[("ws", l)]
        for r in range(reps):
            p0 = r * (64 if reps == 2 else 32)
            w16[p0 : p0 + kin, col : col + kout] = ws[l].astype(ml_dtypes.bfloat16)

    in_maps = []
    for c in range(N_CORES):
        x_c = x[c * BC : (c + 1) * BC, :]
        # [128, NT*4*NB]: tile n at col n*2048, feature chunk ki at +ki*512
        xt_c = np.ascontiguousarray(
            x_c.reshape(NT, NB, 4, 128).transpose(3, 0, 2, 1).reshape(128, -1)
        ).astype(np_in)
        in_maps.append({"xt": xt_c, "wblob": wblob, "bblob": bblob, "w16": w16})
    return in_maps


def kernel(**inputs):
    x = np.asarray(inputs["x"], dtype=np.float32)
    ws = [np.asarray(inputs[f"w{i}"], dtype=np.float32) for i in range(N_LAYERS)]
    bs = [np.asarray(inputs[f"b{i}"], dtype=np.float32) for i in range(N_LAYERS)]

    nc = _get_nc()
    in_maps = prep_in_maps(x, ws, bs)
    res = run_bass_kernel_spmd(nc, in_maps, core_ids=list(range(N_CORES)))
    out = np.concatenate([res.results[c]["out"] for c in range(N_CORES)], axis=0)
    return out.astype(np.float32)
